# revision 25
# baseline (speedup 1.0000x reference)
"""Trainium2 Bass kernel for GQA attention with RoPE (nn_Attention_21603685499660).

Shapes (hardcoded): x [2, 2048, 4096], H=32 Q heads, KVH=8 KV heads, HD=128.
Sharding over 8 NeuronCores: core c -> batch b = c//4, head-group g = c%4
(8 Q heads, 2 KV heads per core).  Each core computes a partial output
(its heads' attention output through its slice of wo); the host sums the
4 partials per batch.  No on-device collectives.

Per-core pipeline (all matmuls bf16 with f32 PSUM accumulation):
  1. QKV projection from host-pre-transposed x and weights.  Q/K are
     produced directly in transposed [HD, seq] layout; V in natural
     [seq, HD] layout.  RoPE applied via a rotate-half permutation
     matmul (head dims de-interleaved host-side).  For the causal case,
     attention chunk 0 (k-tiles 0-3, available after stage-1 chunk 0)
     is interleaved into stage-1 chunks 1-3 — one QK/exp/PV tile
     iteration after each projection group — so its ACT-bound work
     rides under stage-1's dense PE stream instead of stalling the PE
     (and HAM-rethrottling the clock) at stage-2 entry.  Its pair-sums
     are stored (qds) and its softmax denominators finalized at stage-2
     entry.
  2. Attention with scores computed transposed: ST[k,q] = K @ Q^T per
     (head, 512-wide q chunk, 128-wide k tile).  Softmax without max
     subtraction (scores are O(+-10)): P = exp(ST); causal masking is
     multiplicative 0/1 on the P tile in bf16, and the diagonal blocks
     compute QK/exp/PV only on their unmasked column sub-range
     (q >= 128*i for pattern i).  The denominator l accumulates on
     TensorE into an [8,512] PSUM tile (one-hot stationary routes the
     column sum to rows h and h+4; the mirror keeps unused rows finite)
     split heads 0-3 / 4-7 per chunk; 1/l is one DVE
     reciprocal_approx_fast per half (no Ln -> single activation table
     set for the whole kernel).  1/l row h is partition-broadcast by a
     one-hot-row matmul; raw PV output is evacuated on ScalarE and
     normalized in place once its half's reciprocal is ready.
  3. Output projection po[q,n] += attnT[d,q]^T @ woT[d,n], emitted as
     PE filler interleaved into the NEXT chunk's attention.  wo is
     streamed nn-major and pending groups are consumed nn-major so the
     first groups never wait on the 8.4MB wo DMA.
"""

from contextlib import ExitStack

import numpy as np
import ml_dtypes

import concourse.bass as bass
import concourse.tile as tile
from concourse import bacc, mybir
from concourse.bass_utils import run_bass_kernel_spmd

B, S, D = 2, 2048, 4096
H, KVH, HD = 32, 8, 128
N_CORES = 8
GROUPS = 4            # head groups (tensor-parallel dim); B * GROUPS = 8 cores
HL = H // GROUPS      # 8 local Q heads
KVL = KVH // GROUPS   # 2 local KV heads
FQK = HL + KVL        # 10 feature tiles of 128 (Q heads then K heads)
NJ = S // 512         # 4 seq chunks of 512
NT = S // 128         # 16 seq tiles of 128
ND = D // 128         # 32 contraction tiles
BF = mybir.dt.bfloat16
F32 = mybir.dt.float32
EXP = mybir.ActivationFunctionType.Exp

_BUILD_CACHE: dict = {}


def _build(mask_mode: str):
    """mask_mode: 'causal' | 'zero' | 'general'."""
    causal = mask_mode == "causal"
    nc = bacc.Bacc("TRN2", target_bir_lowering=False, debug=False,
                   num_devices=N_CORES)

    xt_d = nc.dram_tensor("xt", [128, ND, S], BF, kind="ExternalInput").ap()
    wqk_d = nc.dram_tensor("wqk", [FQK, 128, ND, 128], BF, kind="ExternalInput").ap()
    wv_d = nc.dram_tensor("wv", [128, ND, KVL * HD], BF, kind="ExternalInput").ap()
    wo_d = nc.dram_tensor("wo", [128, HL, D], BF, kind="ExternalInput").ap()
    cos_d = nc.dram_tensor("cosd", [128, S], BF, kind="ExternalInput").ap()
    sin_d = nc.dram_tensor("sind", [128, S], BF, kind="ExternalInput").ap()
    pm_d = nc.dram_tensor("pm", [128, 128], BF, kind="ExternalInput").ap()
    oneh_d = nc.dram_tensor("oneh", [128, HL, HL], BF, kind="ExternalInput").ap()
    oneb_d = nc.dram_tensor("oneb", [HL, HL, 128], BF, kind="ExternalInput").ap()
    if causal:
        # the 4 diagonal-block 0/1 patterns are chunk-invariant
        mk_d = nc.dram_tensor("maskd", [4, 128, 512], BF, kind="ExternalInput").ap()
    elif mask_mode == "general":
        mk_d = nc.dram_tensor("maskt", [S, S], BF, kind="ExternalInput").ap()
    po_d = nc.dram_tensor("po", [S, D], F32, kind="ExternalOutput").ap()

    def apply_tiles(j):
        if causal:
            return list(range(4 * j, 4 * j + 4))
        if mask_mode == "general":
            return list(range(NT))
        return []

    with tile.TileContext(nc) as tc, ExitStack() as ctx:
        resident = ctx.enter_context(tc.tile_pool(name="resident", bufs=1))
        qkv = ctx.enter_context(tc.tile_pool(name="qkv", bufs=1))

        ones_col = resident.tile([128, 1], BF)
        nc.vector.memset(ones_col[:], 1.0)
        pmb = resident.tile([128, 128], BF)
        nc.sync.dma_start(out=pmb[:], in_=pm_d[:])
        onehb = resident.tile([128, HL, HL], BF)   # l-accum one-hot columns
        nc.sync.dma_start(out=onehb[:], in_=oneh_d[:])
        onebb = resident.tile([HL, HL, 128], BF)   # 1/l bcast one-hot rows
        nc.sync.dma_start(out=onebb[:], in_=oneb_d[:])
        if causal:
            mskr = resident.tile([128, 4, 512], BF)
            for i in range(4):
                nc.sync.dma_start(out=mskr[:, i, :], in_=mk_d[i])
        # preload the Exp activation table set while the first DMAs land;
        # no other set is ever used, so this is the kernel's only
        # ACT_TABLE_LOAD (copy lives in every set)
        dume = resident.tile([128, 1], BF)
        nc.scalar.activation(out=dume[:], in_=ones_col[:], func=EXP)
        wmv = resident.tile([128, 128], BF)   # warm-up moving operand
        nc.vector.memset(wmv[:], 0.0)

        QT = qkv.tile([128, HL, S], BF)    # [HD, head, seq] (de-interleaved)
        KT = qkv.tile([128, KVL, S], BF)
        V = qkv.tile([128, NT, KVL * HD], BF)  # [seq%128, seqtile, kv-head*HD]
        if causal:
            attn0 = qkv.tile([128, HL, 512], BF)   # chunk-0 raw attention out
            qds = qkv.tile([128, HL, 512], BF)     # chunk-0 P quad-sums

        # ---- stage 1: QKV projection + RoPE (+ causal chunk-0 attention) ----
        with tc.tile_pool(name="s1const", bufs=1) as s1const, \
             tc.tile_pool(name="xpool", bufs=2) as xpool, \
             tc.tile_pool(name="wpool", bufs=2) as wpool, \
             tc.tile_pool(name="tpool", bufs=3) as tpool, \
             tc.tile_pool(name="p0pool", bufs=3) as p0pool, \
             tc.tile_pool(name="ps_qk", bufs=2, space="PSUM") as ps_qk, \
             tc.tile_pool(name="ps_ro", bufs=1, space="PSUM") as ps_ro, \
             tc.tile_pool(name="ps_v", bufs=2, space="PSUM") as ps_v, \
             tc.tile_pool(name="ps_c0", bufs=1, space="PSUM") as ps_c0, \
             tc.tile_pool(name="ps_c0o", bufs=1, space="PSUM") as ps_c0o:
            cosb = s1const.tile([128, S], BF)
            sinb = s1const.tile([128, S], BF)
            wvb = s1const.tile([128, ND, KVL * HD], BF)
            # PE warm-up through the HAM window while the first DMAs land;
            # memset moving operand so it depends on no DMA.
            for _ in range(48):
                wtile = ps_ro.tile([1, 128], F32, tag="warm")
                nc.tensor.matmul(wtile[:], ones_col[:], wmv[:], start=True, stop=True)

            # -- causal chunk-0 attention, one tile-iteration per call,
            # interleaved into stage-1 chunks 1-3 (32 iterations total) --
            c0 = {"s": 0, "outp": None, "pts": None, "prs": None}

            def c0_pv(t):
                lo = 128 * t
                nc.tensor.matmul(c0["outp"][:, lo:],
                                 V[:, t, bass.ts(c0["s"] // 4 // 4, 128)],
                                 c0["pts"][t][:, lo:],
                                 start=(t == 0), stop=(t == 3))

            def c0_step():
                s = c0["s"]
                if not causal or s >= 32:
                    return
                h, t = s // 4, s % 4
                hk = h // 4
                lo = 128 * t
                if t == 0:
                    c0["outp"] = ps_c0o.tile([128, 512], F32, tag="o0",
                                             name="outp0")
                    c0["pts"], c0["prs"] = [], []
                stp0 = ps_c0.tile([128, 512], F32, tag="st0", name="stp0")
                nc.tensor.matmul(stp0[:, lo:], KT[:, hk, bass.ts(t, 128)],
                                 QT[:, h, lo:512], start=True, stop=True)
                if t > 0:
                    c0_pv(t - 1)
                pt0 = p0pool.tile([128, 512], BF, tag="pt0")
                if lo:
                    nc.vector.memset(pt0[:, :lo], 0.0)
                nc.scalar.activation(out=pt0[:, lo:], in_=stp0[:, lo:], func=EXP)
                nc.vector.tensor_mul(pt0[:, lo:], pt0[:, lo:], mskr[:, t, lo:])
                c0["pts"].append(pt0)
                if t % 2 == 1:
                    pr = p0pool.tile([128, 512], BF, tag="pr0", bufs=2)
                    nc.vector.tensor_add(pr[:], c0["pts"][t - 1][:],
                                         c0["pts"][t][:])
                    c0["prs"].append(pr)
                if t == 3:
                    nc.vector.tensor_add(qds[:, h, :], c0["prs"][0][:],
                                         c0["prs"][1][:])
                    c0_pv(3)
                    nc.scalar.copy(out=attn0[:, h, :], in_=c0["outp"][:])
                c0["s"] += 1

            def rope_emit(ent):
                # deferred RoPE for a finished projection group: done one
                # group later so its pq matmul / DVE work never stalls PE
                raw, f, js = ent
                pq = ps_ro.tile([128, 512], F32, tag="ro")
                nc.tensor.matmul(pq[:], pmb[:], raw[:], start=True, stop=True)
                t1 = tpool.tile([128, 512], BF, tag="t1")
                nc.vector.tensor_mul(t1[:], raw[:], cosb[:, js])
                t2 = tpool.tile([128, 512], BF, tag="t2")
                nc.vector.tensor_mul(t2[:], pq[:], sinb[:, js])
                dest = QT[:, f, js] if f < HL else KT[:, f - HL, js]
                nc.vector.tensor_add(dest, t1[:], t2[:])

            # chunk 0's x in 4 sub-DMAs (first 8 K-tiles land fast); later
            # chunks prefetched as one DMA during the previous chunk
            wf_next = wpool.tile([128, ND, 128], BF, tag="wf")
            nc.sync.dma_start(out=wf_next[:], in_=wqk_d[0])
            xj_cur = xpool.tile([128, ND, 512], BF, tag="xj", name="xj0")
            for q in range(4):
                nc.sync.dma_start(out=xj_cur[:, bass.ts(q, 8), :],
                                  in_=xt_d[:, bass.ts(q, 8), bass.ts(0, 512)])
            for j in range(NJ):
                js = bass.ts(j, 512)
                xj = xj_cur
                for f in range(FQK):
                    wf = wf_next
                    # prefetch the next group's weights one group ahead
                    nf = f + 1 if f + 1 < FQK else 0
                    if f + 1 < FQK or j + 1 < NJ:
                        wf_next = wpool.tile([128, ND, 128], BF, tag="wf")
                        nc.sync.dma_start(out=wf_next[:], in_=wqk_d[nf])
                    else:
                        wf_next = None
                    if j == 0 and f == 0:
                        nc.sync.dma_start(out=cosb[:], in_=cos_d[:])
                        nc.sync.dma_start(out=sinb[:], in_=sin_d[:])
                    if j == 0 and f in (6, 7):
                        # wv halves late enough not to delay the wf
                        # prefetch chain; needed only by the j=0 V groups
                        half = bass.ts(f - 6, ND // 2)
                        nc.sync.dma_start(out=wvb[:, half, :],
                                          in_=wv_d[:, half, :])
                    if f in (1, 4) and j + 1 < NJ:
                        # next chunk's x in two halves so the wf prefetches
                        # in between are not delayed behind 4.2MB
                        if f == 1:
                            xj_cur = xpool.tile([128, ND, 512], BF, tag="xj",
                                                name="xjn")
                        half = bass.ts(f // 3, ND // 2)
                        nc.sync.dma_start(
                            out=xj_cur[:, half, :],
                            in_=xt_d[:, half, bass.ts(j + 1, 512)])
                    ps = ps_qk.tile([128, 512], F32, tag="qk")
                    for n in range(ND):
                        nc.tensor.matmul(ps[:], wf[:, n, :], xj[:, n, :],
                                         start=(n == 0), stop=(n == ND - 1))
                    raw = tpool.tile([128, 512], BF, tag="raw")
                    nc.scalar.copy(out=raw[:], in_=ps[:])
                    rope_emit((raw, f, js))
                    if j >= 1:
                        c0_step()
                for tt in range(4):
                    psv = ps_v.tile([128, KVL * HD], F32, tag="v")
                    for n in range(ND):
                        nc.tensor.matmul(psv[:], xj[:, n, bass.ts(tt, 128)],
                                         wvb[:, n, :],
                                         start=(n == 0), stop=(n == ND - 1))
                    nc.scalar.copy(out=V[:, j * 4 + tt, :], in_=psv[:])
                    if j >= 1:
                        c0_step()

        # attnT + wo live from stage 2 through stage 3 (pool opened only now
        # so stage 1 had the SBUF).
        att_out = ctx.enter_context(tc.tile_pool(name="att_out", bufs=1))
        attnT = att_out.tile([128, HL, S], BF)  # [HD, head, seq]
        wob = att_out.tile([128, HL, D], BF)

        # ---- stage 2+3: attention chunks (1-3 for causal) + out-projection --
        po_state = {"cur": None, "dd": 0}

        def po_step(budget):
            # emit up to `budget` output-projection matmuls as PE filler
            for _ in range(budget):
                if po_state["cur"] is None:
                    if not pending_po:
                        return
                    qt, nn = pending_po.pop(0)
                    pop = ps_po.tile([128, 512], F32, tag="po", name="pop")
                    po_state["cur"] = (qt, nn, pop)
                    po_state["dd"] = 0
                qt, nn, pop = po_state["cur"]
                dd = po_state["dd"]
                src = (attn0[:, dd, bass.ts(qt, 128)] if causal and qt < 4
                       else attnT[:, dd, bass.ts(qt, 128)])
                nc.tensor.matmul(pop[:], src, wob[:, dd, bass.ts(nn, 512)],
                                 start=(dd == 0), stop=(dd == HL - 1))
                po_state["dd"] += 1
                if po_state["dd"] == HL:
                    stg = spool.tile([128, 512], F32, tag="stg")
                    # alternate evacuation engine to balance ACT/DVE load
                    if (qt + nn) % 2:
                        nc.scalar.copy(out=stg[:], in_=pop[:])
                    else:
                        nc.vector.tensor_copy(stg[:], pop[:])
                    nc.sync.dma_start(
                        out=po_d[bass.ts(qt, 128), bass.ts(nn, 512)], in_=stg[:])
                    po_state["cur"] = None

        with tc.tile_pool(name="mpool", bufs=1) as mpool, \
             tc.tile_pool(name="ppool", bufs=6) as ppool, \
             tc.tile_pool(name="qpool", bufs=4) as qpool, \
             tc.tile_pool(name="npool", bufs=2) as npool, \
             tc.tile_pool(name="spool", bufs=3) as spool, \
             tc.tile_pool(name="ps_st", bufs=2, space="PSUM") as ps_st, \
             tc.tile_pool(name="ps_o", bufs=2, space="PSUM") as ps_o, \
             tc.tile_pool(name="ps_l", bufs=1, space="PSUM") as ps_l, \
             tc.tile_pool(name="ps_rb", bufs=1, space="PSUM") as ps_rb, \
             tc.tile_pool(name="ps_po", bufs=2, space="PSUM") as ps_po:
            pending_po = []  # (qt, nn) groups ready to emit as PE filler
            lp = {"cur": None}
            rlh = {}

            def emit_recip(g):
                lr32 = npool.tile([HL, 512], F32, tag="lr", name="lr32")
                nc.vector.reciprocal_approx_fast(out=lr32[:], in_=lp["cur"][:])
                rl = npool.tile([HL, 512], BF, tag="rl", name="rl")
                nc.vector.tensor_copy(rl[:], lr32[:])
                rlh[g] = rl
                lp["cur"] = None

            def emit_tail(h, dest):
                # broadcast row h of 1/l across partitions (one-hot row
                # matmul) and normalize the raw attention output in place
                rbp = ps_rb.tile([128, 512], F32, tag="rb", name="rbp")
                nc.tensor.matmul(rbp[:], onebb[:, h, :], rlh[h // 4][:],
                                 start=True, stop=True)
                nc.vector.tensor_mul(dest, dest, rbp[:])

            # wo streamed nn-major so the first (nn-major-ordered) po groups
            # never wait on the full 8.4MB transfer
            for nn in range(D // 512):
                nc.sync.dma_start(out=wob[:, :, bass.ts(nn, 512)],
                                  in_=wo_d[:, :, bass.ts(nn, 512)])

            if causal:
                # finalize chunk 0: denominators from the stored quad-sums,
                # then normalize attn0 and queue its output projection
                for g in range(2):
                    lp["cur"] = ps_l.tile([HL, 512], F32, tag="l", name="lp")
                    for hh in range(4):
                        h = 4 * g + hh
                        nc.tensor.matmul(lp["cur"][:], onehb[:, h, :],
                                         qds[:, h, :],
                                         start=(hh == 0), stop=(hh == 3))
                    emit_recip(g)
                for h in range(HL):
                    emit_tail(h, attn0[:, h, :])
                pending_po.extend(
                    (qt, nn) for nn in range(D // 512) for qt in range(4))

            for j in range(1 if causal else 0, NJ):
                js = bass.ts(j, 512)
                nkt = 4 * (j + 1) if causal else NT
                atiles = apply_tiles(j)
                if causal:
                    msk = mskr
                elif mask_mode == "general":
                    msk = mpool.tile([128, NT, 512], BF, tag="msk")
                    for idx, t in enumerate(atiles):
                        nc.sync.dma_start(out=msk[:, idx, :],
                                          in_=mk_d[bass.ts(t, 128), js])
                # l accumulation groups of 8 k-tiles (remainder 4): pair-sums
                # on DVE as tiles arrive, tree-combine + one l matmul per group
                if nkt <= 8:
                    groups = [(0, nkt)]
                else:
                    groups = [(0, 8), (8, nkt)]
                los = [0] * nkt
                if causal:
                    for i in range(1, 4):
                        los[4 * j + i] = 128 * i
                first_j = j == (1 if causal else 0)
                for h in range(HL):
                    hk = h // (HL // KVL)
                    if h == 4:
                        emit_recip(0)
                    if 4 <= h:
                        # tails for heads 0-3 spread over heads 4-7
                        emit_tail(h - 4, attnT[:, h - 4, js])
                    if lp["cur"] is None:
                        lp["cur"] = ps_l.tile([HL, 512], F32, tag="l",
                                              name="lp")
                    outp = ps_o.tile([128, 512], F32, tag="out")
                    pts = []
                    # software pipeline: PV_t emitted three tiles after QK_t so
                    # QK + filler sit in the PE stream while exp_t (+mask) runs
                    def emit_pv(t):
                        lo = los[t]
                        nc.tensor.matmul(outp[:, lo:], V[:, t, bass.ts(hk, 128)],
                                         pts[t][:, lo:],
                                         start=(t == 0), stop=(t == nkt - 1))

                    gi = 0
                    gpairs = []
                    pogate = 6 if (causal and first_j) else 2
                    for t in range(nkt):
                        # causal diagonal block i only has unmasked columns
                        # >= 128*i: compute QK/exp/PV on that sub-range and
                        # zero the rest of the P tile
                        lo = los[t]
                        stp = ps_st.tile([128, 512], F32, tag="st")
                        nc.tensor.matmul(stp[:, lo:], KT[:, hk, bass.ts(t, 128)],
                                         QT[:, h, j * 512 + lo:(j + 1) * 512],
                                         start=True, stop=True)
                        pt = ppool.tile([128, 512], BF, tag="pt")
                        if lo:
                            # masked-column zeroing on the idle GpSimd engine
                            nc.gpsimd.memset(pt[:, :lo], 0.0)
                        nc.scalar.activation(out=pt[:, lo:], in_=stp[:, lo:],
                                             func=EXP)
                        if t in atiles:
                            # multiplicative mask exp(m): 0/1 for causal
                            idx = atiles.index(t)
                            nc.vector.tensor_mul(
                                pt[:, lo:], pt[:, lo:], msk[:, idx, lo:])
                        pts.append(pt)
                        if (j > 0 or not causal) and t >= pogate:
                            po_step(2)
                        if t > 2:
                            emit_pv(t - 3)
                        if t % 2 == 1:
                            pr = qpool.tile([128, 512], BF, tag="pr", bufs=5)
                            nc.vector.tensor_add(pr[:], pts[t - 1][:],
                                                 pts[t][:])
                            gpairs.append(pr)
                        if t == groups[gi][1] - 1:
                            # tree-combine on GpSimd: off the DVE queue that
                            # the exp->mask->PV chain competes with; the l
                            # matmul consuming the result has tiles of slack
                            while len(gpairs) > 1:
                                nxt = []
                                for k in range(0, len(gpairs) - 1, 2):
                                    u = qpool.tile([128, 512], BF, tag="u",
                                                   bufs=3)
                                    nc.gpsimd.tensor_add(
                                        u[:], gpairs[k][:], gpairs[k + 1][:])
                                    nxt.append(u)
                                if len(gpairs) % 2:
                                    nxt.append(gpairs[-1])
                                gpairs = nxt
                            nc.tensor.matmul(
                                lp["cur"][:], onehb[:, h, :], gpairs[0][:],
                                start=(h % 4 == 0 and gi == 0),
                                stop=(h % 4 == 3 and gi == len(groups) - 1))
                            gpairs = []
                            gi += 1
                    emit_pv(nkt - 3)
                    emit_pv(nkt - 2)
                    emit_pv(nkt - 1)
                    # raw evacuation on ScalarE (PSUM-fast port; DVE carries
                    # the pair-sum + mask load); normalized in place later
                    nc.scalar.copy(out=attnT[:, h, js], in_=outp[:])
                    # PE filler between heads covers the exp pipeline refill
                    po_step(16)
                emit_recip(1)
                for h in range(4, HL):
                    emit_tail(h, attnT[:, h, js])
                pending_po.extend(
                    (qt, nn) for nn in range(D // 512)
                    for qt in range(4 * j, 4 * j + 4))
            while pending_po or po_state["cur"] is not None:
                po_step(8)

    nc.compile()
    return nc


def _get_nc(mask_mode: str):
    if mask_mode not in _BUILD_CACHE:
        _BUILD_CACHE[mask_mode] = _build(mask_mode)
    return _BUILD_CACHE[mask_mode]


_DEINT = np.concatenate([np.arange(0, HD, 2), np.arange(1, HD, 2)])  # de-interleave


def _host_prep(x, freqs_cos, freqs_sin, mask, wq, wk, wv, wo):
    bf16 = ml_dtypes.bfloat16
    scale = float(HD) ** -0.5

    # mask mode
    mask = np.asarray(mask, np.float32)
    tril = np.tril(np.ones((S, S), bool))
    if np.all(mask == 0):
        mask_mode = "zero"
    elif np.all(mask[tril] == 0) and np.all(mask[~tril] <= -1e8):
        mask_mode = "causal"
    else:
        mask_mode = "general"

    # weights: de-interleave head dims of wq/wk; fold softmax scale into wq
    wq_p = (np.asarray(wq, np.float32).reshape(H, HD, D)[:, _DEINT, :] * scale)
    wk_p = np.asarray(wk, np.float32).reshape(KVH, HD, D)[:, _DEINT, :]
    wv_n = np.asarray(wv, np.float32).reshape(KVH, HD, D)
    wo_n = np.asarray(wo, np.float32)

    per_group = []
    for g in range(GROUPS):
        feats = np.concatenate([
            wq_p[g * HL:(g + 1) * HL].reshape(HL * HD, D),
            wk_p[g * KVL:(g + 1) * KVL].reshape(KVL * HD, D),
        ], axis=0)  # [1280, D]
        wqk_dma = np.ascontiguousarray(
            feats.reshape(FQK, 128, ND, 128).transpose(0, 3, 2, 1)).astype(bf16)
        wvg = wv_n[g * KVL:(g + 1) * KVL].reshape(KVL * HD, D)
        wv_dma = np.ascontiguousarray(
            wvg.reshape(KVL * HD, ND, 128).transpose(2, 1, 0)).astype(bf16)
        woT = wo_n[:, g * HL * HD:(g + 1) * HL * HD].T  # [1024, D]
        wo_dma = np.ascontiguousarray(
            woT.reshape(HL, 128, D).transpose(1, 0, 2)).astype(bf16)
        per_group.append((wqk_dma, wv_dma, wo_dma))

    xs = []
    for b in range(B):
        xT = np.asarray(x[b], np.float32).T  # [D, S]
        xs.append(np.ascontiguousarray(
            xT.reshape(ND, 128, S).transpose(1, 0, 2)).astype(bf16))

    cosT = np.asarray(freqs_cos, np.float32).T  # [64, S]
    sinT = np.asarray(freqs_sin, np.float32).T
    cos_dma = np.ascontiguousarray(np.concatenate([cosT, cosT], 0)).astype(bf16)
    sin_dma = np.ascontiguousarray(np.concatenate([sinT, sinT], 0)).astype(bf16)

    P = np.zeros((128, 128), np.float32)
    for r in range(64):
        P[r, 64 + r] = -1.0
        P[64 + r, r] = 1.0
    pm = np.ascontiguousarray(P.T).astype(bf16)

    # one-hot helpers for the softmax denominator: oneh[:, h, :] routes the
    # column sum into rows h and (h+4)%8 — the mirror keeps the half-chunk
    # accumulator's unused rows finite (reciprocal of an exact 0 row would
    # make NaN/inf that the 0-weights of the broadcast matmul still absorb
    # as 0*inf=NaN); oneb[:, h, :] has row h all-ones (broadcast 1/l row h
    # across partitions)
    eye44 = np.tile(np.eye(4, dtype=np.float32), (2, 2))  # m ≡ h (mod 4)
    oneh = np.ascontiguousarray(
        np.broadcast_to(eye44, (128, HL, HL))).astype(bf16)
    oneb = np.ascontiguousarray(
        np.broadcast_to(np.eye(HL, dtype=np.float32)[:, :, None],
                        (HL, HL, 128))).astype(bf16)

    # mask is applied multiplicatively after exp: P *= exp(mask)
    mask_extra = {}
    if mask_mode == "causal":
        # diagonal-block patterns are chunk-invariant: block (t=4j+i, j)
        # only depends on i
        mT = np.exp(np.minimum(mask.T, 0.0))
        md = np.empty((4, 128, 512), np.float32)
        for i in range(4):
            md[i] = mT[i * 128:(i + 1) * 128, 0:512]
        mask_extra["maskd"] = md.astype(bf16)
    elif mask_mode == "general":
        with np.errstate(over="ignore"):
            mask_extra["maskt"] = np.ascontiguousarray(
                np.exp(mask.T)).astype(bf16)

    in_maps = []
    for c in range(N_CORES):
        b, g = c // GROUPS, c % GROUPS
        wqk_dma, wv_dma, wo_dma = per_group[g]
        m = {"xt": xs[b], "wqk": wqk_dma, "wv": wv_dma, "wo": wo_dma,
             "cosd": cos_dma, "sind": sin_dma, "pm": pm,
             "oneh": oneh, "oneb": oneb}
        m.update(mask_extra)
        in_maps.append(m)
    return mask_mode, in_maps


def kernel(x, freqs_cos, freqs_sin, positions, mask, wq, wk, wv, wo,
           _want_profile=False):
    mask_mode, in_maps = _host_prep(x, freqs_cos, freqs_sin, mask, wq, wk, wv, wo)
    nc = _get_nc(mask_mode)
    res = run_bass_kernel_spmd(nc, in_maps, core_ids=list(range(N_CORES)),
                               trace=_want_profile)
    out = np.zeros((B, S, D), np.float32)
    for c in range(N_CORES):
        out[c // GROUPS] += res.results[c]["po"]
    if _want_profile:
        kernel.last_exec_time_ns = res.exec_time_ns
        kernel.last_results = res
    return out


# revision 26
# speedup vs baseline: 1.0651x; 1.0651x over previous
"""Trainium2 Bass kernel for GQA attention with RoPE (nn_Attention_21603685499660).

Shapes (hardcoded): x [2, 2048, 4096], H=32 Q heads, KVH=8 KV heads, HD=128.
Sharding over 8 NeuronCores: core c -> batch b = c//4, head-group g = c%4
(8 Q heads, 2 KV heads per core).  Each core computes a partial output
(its heads' attention output through its slice of wo); the host sums the
4 partials per batch.  No on-device collectives.

Per-core pipeline (all matmuls bf16 with f32 PSUM accumulation):
  1. QKV projection from host-pre-transposed x and weights.  Q/K are
     produced directly in transposed [HD, seq] layout; V in natural
     [seq, HD] layout.  RoPE applied via a rotate-half permutation
     matmul (head dims de-interleaved host-side).  For the causal case,
     attention chunk 0 (k-tiles 0-3, available after stage-1 chunk 0)
     is interleaved into stage-1 chunks 1-3 — one QK/exp/PV tile
     iteration after each projection group — so its ACT-bound work
     rides under stage-1's dense PE stream instead of stalling the PE
     (and HAM-rethrottling the clock) at stage-2 entry.  Its pair-sums
     are stored (qds) and its softmax denominators finalized at stage-2
     entry.
  2. Attention with scores computed transposed: ST[k,q] = K @ Q^T per
     (head, 512-wide q chunk, 128-wide k tile).  Softmax without max
     subtraction (scores are O(+-10)): P = exp(ST); causal masking is
     multiplicative 0/1 on the P tile in bf16, and the diagonal blocks
     compute QK/exp/PV only on their unmasked column sub-range
     (q >= 128*i for pattern i).  The denominator l accumulates on
     TensorE into an [8,512] PSUM tile (one-hot stationary routes the
     column sum to rows h and h+4; the mirror keeps unused rows finite)
     split heads 0-3 / 4-7 per chunk; 1/l is one DVE
     reciprocal_approx_fast per half (no Ln -> single activation table
     set for the whole kernel).  1/l row h is partition-broadcast by a
     one-hot-row matmul; raw PV output is evacuated on ScalarE and
     normalized in place once its half's reciprocal is ready.
  3. Output projection po[q,n] += attnT[d,q]^T @ woT[d,n], emitted as
     PE filler interleaved into the NEXT chunk's attention.  wo is
     streamed nn-major and pending groups are consumed nn-major so the
     first groups never wait on the 8.4MB wo DMA.
"""

from contextlib import ExitStack

import numpy as np
import ml_dtypes

import concourse.bass as bass
import concourse.tile as tile
from concourse import bacc, mybir
from concourse.bass_utils import run_bass_kernel_spmd

B, S, D = 2, 2048, 4096
H, KVH, HD = 32, 8, 128
N_CORES = 8
GROUPS = 4            # head groups (tensor-parallel dim); B * GROUPS = 8 cores
HL = H // GROUPS      # 8 local Q heads
KVL = KVH // GROUPS   # 2 local KV heads
FQK = HL + KVL        # 10 feature tiles of 128 (Q heads then K heads)
NJ = S // 512         # 4 seq chunks of 512
NT = S // 128         # 16 seq tiles of 128
ND = D // 128         # 32 contraction tiles
BF = mybir.dt.bfloat16
F32 = mybir.dt.float32
EXP = mybir.ActivationFunctionType.Exp

_BUILD_CACHE: dict = {}


def _build(mask_mode: str):
    """mask_mode: 'causal' | 'zero' | 'general'."""
    causal = mask_mode == "causal"
    nc = bacc.Bacc("TRN2", target_bir_lowering=False, debug=False,
                   num_devices=N_CORES)

    xt_d = nc.dram_tensor("xt", [128, ND, S], BF, kind="ExternalInput").ap()
    wqk_d = nc.dram_tensor("wqk", [FQK, 128, ND, 128], BF, kind="ExternalInput").ap()
    wv_d = nc.dram_tensor("wv", [128, ND, KVL * HD], BF, kind="ExternalInput").ap()
    wo_d = nc.dram_tensor("wo", [128, HL, D], BF, kind="ExternalInput").ap()
    cos_d = nc.dram_tensor("cosd", [128, S], BF, kind="ExternalInput").ap()
    sin_d = nc.dram_tensor("sind", [128, S], BF, kind="ExternalInput").ap()
    pm_d = nc.dram_tensor("pm", [128, 128], BF, kind="ExternalInput").ap()
    oneh_d = nc.dram_tensor("oneh", [128, HL, HL], BF, kind="ExternalInput").ap()
    oneb_d = nc.dram_tensor("oneb", [HL, HL, 128], BF, kind="ExternalInput").ap()
    if causal:
        # the 4 diagonal-block 0/1 patterns are chunk-invariant
        mk_d = nc.dram_tensor("maskd", [4, 128, 512], BF, kind="ExternalInput").ap()
    elif mask_mode == "general":
        mk_d = nc.dram_tensor("maskt", [S, S], BF, kind="ExternalInput").ap()
    po_d = nc.dram_tensor("po", [S, D], F32, kind="ExternalOutput").ap()

    def apply_tiles(j):
        if causal:
            return list(range(4 * j, 4 * j + 4))
        if mask_mode == "general":
            return list(range(NT))
        return []

    with tile.TileContext(nc) as tc, ExitStack() as ctx:
        resident = ctx.enter_context(tc.tile_pool(name="resident", bufs=1))
        qkv = ctx.enter_context(tc.tile_pool(name="qkv", bufs=1))

        ones_col = resident.tile([128, 1], BF)
        nc.vector.memset(ones_col[:], 1.0)
        pmb = resident.tile([128, 128], BF)
        nc.sync.dma_start(out=pmb[:], in_=pm_d[:])
        onehb = resident.tile([128, HL, HL], BF)   # l-accum one-hot columns
        nc.sync.dma_start(out=onehb[:], in_=oneh_d[:])
        onebb = resident.tile([HL, HL, 128], BF)   # 1/l bcast one-hot rows
        nc.sync.dma_start(out=onebb[:], in_=oneb_d[:])
        if causal:
            mskr = resident.tile([128, 4, 512], BF)
            for i in range(4):
                nc.sync.dma_start(out=mskr[:, i, :], in_=mk_d[i])
        # preload the Exp activation table set while the first DMAs land;
        # no other set is ever used, so this is the kernel's only
        # ACT_TABLE_LOAD (copy lives in every set)
        dume = resident.tile([128, 1], BF)
        nc.scalar.activation(out=dume[:], in_=ones_col[:], func=EXP)
        wmv = resident.tile([128, 128], BF)   # warm-up moving operand
        nc.vector.memset(wmv[:], 0.0)

        QT = qkv.tile([128, HL, S], BF)    # [HD, head, seq] (de-interleaved)
        KT = qkv.tile([128, KVL, S], BF)
        V = qkv.tile([128, NT, KVL * HD], BF)  # [seq%128, seqtile, kv-head*HD]
        if causal:
            attn0 = qkv.tile([128, HL, 512], BF)   # chunk-0 raw attention out
            qds = qkv.tile([128, HL, 512], BF)     # chunk-0 P quad-sums

        # ---- stage 1: QKV projection + RoPE (+ causal chunk-0 attention) ----
        with tc.tile_pool(name="s1const", bufs=1) as s1const, \
             tc.tile_pool(name="xpool", bufs=2) as xpool, \
             tc.tile_pool(name="wpool", bufs=2) as wpool, \
             tc.tile_pool(name="tpool", bufs=3) as tpool, \
             tc.tile_pool(name="p0pool", bufs=3) as p0pool, \
             tc.tile_pool(name="ps_qk", bufs=2, space="PSUM") as ps_qk, \
             tc.tile_pool(name="ps_ro", bufs=1, space="PSUM") as ps_ro, \
             tc.tile_pool(name="ps_v", bufs=2, space="PSUM") as ps_v, \
             tc.tile_pool(name="ps_c0", bufs=1, space="PSUM") as ps_c0, \
             tc.tile_pool(name="ps_c0o", bufs=1, space="PSUM") as ps_c0o:
            cosb = s1const.tile([128, S], BF)
            sinb = s1const.tile([128, S], BF)
            wvb = s1const.tile([128, ND, KVL * HD], BF)
            # PE warm-up through the HAM window while the first DMAs land;
            # memset moving operand so it depends on no DMA.
            for _ in range(48):
                wtile = ps_ro.tile([1, 128], F32, tag="warm")
                nc.tensor.matmul(wtile[:], ones_col[:], wmv[:], start=True, stop=True)

            # -- causal chunk-0 attention, one tile-iteration per call,
            # interleaved into stage-1 chunks 1-3 (32 iterations total) --
            c0 = {"s": 0, "outp": None, "pts": None, "prs": None}

            def c0_pv(t):
                lo = 128 * t
                nc.tensor.matmul(c0["outp"][:, lo:],
                                 V[:, t, bass.ts(c0["s"] // 4 // 4, 128)],
                                 c0["pts"][t][:, lo:],
                                 start=(t == 0), stop=(t == 3))

            def c0_step():
                s = c0["s"]
                if not causal or s >= 32:
                    return
                h, t = s // 4, s % 4
                hk = h // 4
                lo = 128 * t
                if t == 0:
                    c0["outp"] = ps_c0o.tile([128, 512], F32, tag="o0",
                                             name="outp0")
                    c0["pts"], c0["prs"] = [], []
                stp0 = ps_c0.tile([128, 512], F32, tag="st0", name="stp0")
                nc.tensor.matmul(stp0[:, lo:], KT[:, hk, bass.ts(t, 128)],
                                 QT[:, h, lo:512], start=True, stop=True)
                if t > 0:
                    c0_pv(t - 1)
                pt0 = p0pool.tile([128, 512], BF, tag="pt0")
                if lo:
                    nc.vector.memset(pt0[:, :lo], 0.0)
                nc.scalar.activation(out=pt0[:, lo:], in_=stp0[:, lo:], func=EXP)
                nc.vector.tensor_mul(pt0[:, lo:], pt0[:, lo:], mskr[:, t, lo:])
                c0["pts"].append(pt0)
                if t % 2 == 1:
                    pr = p0pool.tile([128, 512], BF, tag="pr0", bufs=2)
                    nc.vector.tensor_add(pr[:], c0["pts"][t - 1][:],
                                         c0["pts"][t][:])
                    c0["prs"].append(pr)
                if t == 3:
                    nc.vector.tensor_add(qds[:, h, :], c0["prs"][0][:],
                                         c0["prs"][1][:])
                    c0_pv(3)
                    nc.scalar.copy(out=attn0[:, h, :], in_=c0["outp"][:])
                c0["s"] += 1

            def rope_emit(ent):
                # deferred RoPE for a finished projection group: done one
                # group later so its pq matmul / DVE work never stalls PE
                raw, f, js = ent
                pq = ps_ro.tile([128, 512], F32, tag="ro")
                nc.tensor.matmul(pq[:], pmb[:], raw[:], start=True, stop=True)
                t1 = tpool.tile([128, 512], BF, tag="t1")
                nc.vector.tensor_mul(t1[:], raw[:], cosb[:, js])
                t2 = tpool.tile([128, 512], BF, tag="t2")
                nc.vector.tensor_mul(t2[:], pq[:], sinb[:, js])
                dest = QT[:, f, js] if f < HL else KT[:, f - HL, js]
                nc.vector.tensor_add(dest, t1[:], t2[:])

            # chunk 0's x in 4 sub-DMAs (first 8 K-tiles land fast); later
            # chunks prefetched as one DMA during the previous chunk
            wf_next = wpool.tile([128, ND, 128], BF, tag="wf")
            nc.sync.dma_start(out=wf_next[:], in_=wqk_d[0])
            xj_cur = xpool.tile([128, ND, 512], BF, tag="xj", name="xj0")
            for q in range(4):
                nc.sync.dma_start(out=xj_cur[:, bass.ts(q, 8), :],
                                  in_=xt_d[:, bass.ts(q, 8), bass.ts(0, 512)])
            for j in range(NJ):
                js = bass.ts(j, 512)
                xj = xj_cur
                for f in range(FQK):
                    wf = wf_next
                    # prefetch the next group's weights one group ahead
                    nf = f + 1 if f + 1 < FQK else 0
                    if f + 1 < FQK or j + 1 < NJ:
                        wf_next = wpool.tile([128, ND, 128], BF, tag="wf")
                        nc.sync.dma_start(out=wf_next[:], in_=wqk_d[nf])
                    else:
                        wf_next = None
                    if j == 0 and f == 0:
                        nc.sync.dma_start(out=cosb[:], in_=cos_d[:])
                        nc.sync.dma_start(out=sinb[:], in_=sin_d[:])
                    if j == 0 and f in (6, 7):
                        # wv halves late enough not to delay the wf
                        # prefetch chain; needed only by the j=0 V groups
                        half = bass.ts(f - 6, ND // 2)
                        nc.sync.dma_start(out=wvb[:, half, :],
                                          in_=wv_d[:, half, :])
                    if f in (1, 4) and j + 1 < NJ:
                        # next chunk's x in two halves so the wf prefetches
                        # in between are not delayed behind 4.2MB
                        if f == 1:
                            xj_cur = xpool.tile([128, ND, 512], BF, tag="xj",
                                                name="xjn")
                        half = bass.ts(f // 3, ND // 2)
                        nc.sync.dma_start(
                            out=xj_cur[:, half, :],
                            in_=xt_d[:, half, bass.ts(j + 1, 512)])
                    ps = ps_qk.tile([128, 512], F32, tag="qk")
                    for n in range(ND):
                        nc.tensor.matmul(ps[:], wf[:, n, :], xj[:, n, :],
                                         start=(n == 0), stop=(n == ND - 1))
                    raw = tpool.tile([128, 512], BF, tag="raw")
                    nc.scalar.copy(out=raw[:], in_=ps[:])
                    rope_emit((raw, f, js))
                    if j >= 1:
                        c0_step()
                for tt in range(4):
                    psv = ps_v.tile([128, KVL * HD], F32, tag="v")
                    for n in range(ND):
                        nc.tensor.matmul(psv[:], xj[:, n, bass.ts(tt, 128)],
                                         wvb[:, n, :],
                                         start=(n == 0), stop=(n == ND - 1))
                    nc.scalar.copy(out=V[:, j * 4 + tt, :], in_=psv[:])
                    if j >= 1:
                        c0_step()

        # attnT + wo live from stage 2 through stage 3 (pool opened only now
        # so stage 1 had the SBUF).
        att_out = ctx.enter_context(tc.tile_pool(name="att_out", bufs=1))
        attnT = att_out.tile([128, HL, S], BF)  # [HD, head, seq]
        wob = att_out.tile([128, HL, D], BF)

        # ---- stage 2+3: attention chunks (1-3 for causal) + out-projection --
        po_state = {"cur": None, "dd": 0}

        def po_step(budget):
            # emit up to `budget` output-projection matmuls as PE filler
            for _ in range(budget):
                if po_state["cur"] is None:
                    if not pending_po:
                        return
                    qt, nn = pending_po.pop(0)
                    pop = ps_po.tile([128, 512], F32, tag="po", name="pop")
                    po_state["cur"] = (qt, nn, pop)
                    po_state["dd"] = 0
                qt, nn, pop = po_state["cur"]
                dd = po_state["dd"]
                src = (attn0[:, dd, bass.ts(qt, 128)] if causal and qt < 4
                       else attnT[:, dd, bass.ts(qt, 128)])
                nc.tensor.matmul(pop[:], src, wob[:, dd, bass.ts(nn, 512)],
                                 start=(dd == 0), stop=(dd == HL - 1))
                po_state["dd"] += 1
                if po_state["dd"] == HL:
                    stg = spool.tile([128, 512], F32, tag="stg")
                    # alternate evacuation engine to balance ACT/DVE load
                    if (qt + nn) % 2:
                        nc.scalar.copy(out=stg[:], in_=pop[:])
                    else:
                        nc.vector.tensor_copy(stg[:], pop[:])
                    nc.sync.dma_start(
                        out=po_d[bass.ts(qt, 128), bass.ts(nn, 512)], in_=stg[:])
                    po_state["cur"] = None

        with tc.tile_pool(name="mpool", bufs=1) as mpool, \
             tc.tile_pool(name="ppool", bufs=6) as ppool, \
             tc.tile_pool(name="qpool", bufs=4) as qpool, \
             tc.tile_pool(name="npool", bufs=2) as npool, \
             tc.tile_pool(name="spool", bufs=3) as spool, \
             tc.tile_pool(name="ps_st", bufs=2, space="PSUM") as ps_st, \
             tc.tile_pool(name="ps_o", bufs=2, space="PSUM") as ps_o, \
             tc.tile_pool(name="ps_l", bufs=1, space="PSUM") as ps_l, \
             tc.tile_pool(name="ps_rb", bufs=1, space="PSUM") as ps_rb, \
             tc.tile_pool(name="ps_po", bufs=2, space="PSUM") as ps_po:
            pending_po = []  # (qt, nn) groups ready to emit as PE filler
            lp = {"cur": None}
            rlh = {}

            def emit_recip(g):
                lr32 = npool.tile([HL, 512], F32, tag="lr", name="lr32")
                nc.vector.reciprocal_approx_fast(out=lr32[:], in_=lp["cur"][:])
                rl = npool.tile([HL, 512], BF, tag="rl", name="rl")
                nc.vector.tensor_copy(rl[:], lr32[:])
                rlh[g] = rl
                lp["cur"] = None

            def emit_tail(h, dest):
                # broadcast row h of 1/l across partitions (one-hot row
                # matmul) and normalize the raw attention output in place
                rbp = ps_rb.tile([128, 512], F32, tag="rb", name="rbp")
                nc.tensor.matmul(rbp[:], onebb[:, h, :], rlh[h // 4][:],
                                 start=True, stop=True)
                nc.vector.tensor_mul(dest, dest, rbp[:])

            # wo streamed nn-major so the first (nn-major-ordered) po groups
            # never wait on the full 8.4MB transfer
            for nn in range(D // 512):
                nc.sync.dma_start(out=wob[:, :, bass.ts(nn, 512)],
                                  in_=wo_d[:, :, bass.ts(nn, 512)])

            if causal:
                # finalize chunk 0: denominators from the stored quad-sums,
                # then normalize attn0 and queue its output projection
                for g in range(2):
                    lp["cur"] = ps_l.tile([HL, 512], F32, tag="l", name="lp")
                    for hh in range(4):
                        h = 4 * g + hh
                        nc.tensor.matmul(lp["cur"][:], onehb[:, h, :],
                                         qds[:, h, :],
                                         start=(hh == 0), stop=(hh == 3))
                    emit_recip(g)
                for h in range(HL):
                    emit_tail(h, attn0[:, h, :])
                pending_po.extend(
                    (qt, nn) for nn in range(D // 512) for qt in range(4))

            for j in range(1 if causal else 0, NJ):
                js = bass.ts(j, 512)
                nkt = 4 * (j + 1) if causal else NT
                atiles = apply_tiles(j)
                if causal:
                    msk = mskr
                elif mask_mode == "general":
                    msk = mpool.tile([128, NT, 512], BF, tag="msk")
                    for idx, t in enumerate(atiles):
                        nc.sync.dma_start(out=msk[:, idx, :],
                                          in_=mk_d[bass.ts(t, 128), js])
                # l accumulation groups of 8 k-tiles (remainder 4): pair-sums
                # on DVE as tiles arrive, tree-combine + one l matmul per group
                if nkt <= 8:
                    groups = [(0, nkt)]
                else:
                    groups = [(0, 8), (8, nkt)]
                los = [0] * nkt
                if causal:
                    for i in range(1, 4):
                        los[4 * j + i] = 128 * i
                first_j = j == (1 if causal else 0)
                for h in range(HL):
                    hk = h // (HL // KVL)
                    if h == 4:
                        emit_recip(0)
                    if 4 <= h:
                        # tails for heads 0-3 spread over heads 4-7
                        emit_tail(h - 4, attnT[:, h - 4, js])
                    if lp["cur"] is None:
                        lp["cur"] = ps_l.tile([HL, 512], F32, tag="l",
                                              name="lp")
                    outp = ps_o.tile([128, 512], F32, tag="out")
                    pts = []
                    # software pipeline: PV_t emitted three tiles after QK_t so
                    # QK + filler sit in the PE stream while exp_t (+mask) runs
                    def emit_pv(t):
                        lo = los[t]
                        nc.tensor.matmul(outp[:, lo:], V[:, t, bass.ts(hk, 128)],
                                         pts[t][:, lo:],
                                         start=(t == 0), stop=(t == nkt - 1))

                    gi = 0
                    gpairs = []
                    pogate = 6 if (causal and first_j) else 2
                    for t in range(nkt):
                        # causal diagonal block i only has unmasked columns
                        # >= 128*i: compute QK/exp/PV on that sub-range and
                        # zero the rest of the P tile
                        lo = los[t]
                        stp = ps_st.tile([128, 512], F32, tag="st")
                        nc.tensor.matmul(stp[:, lo:], KT[:, hk, bass.ts(t, 128)],
                                         QT[:, h, j * 512 + lo:(j + 1) * 512],
                                         start=True, stop=True)
                        pt = ppool.tile([128, 512], BF, tag="pt")
                        if lo:
                            # masked-column zeroing on the idle GpSimd engine
                            nc.gpsimd.memset(pt[:, :lo], 0.0)
                        nc.scalar.activation(out=pt[:, lo:], in_=stp[:, lo:],
                                             func=EXP)
                        if t in atiles:
                            # multiplicative mask exp(m): 0/1 for causal
                            idx = atiles.index(t)
                            nc.vector.tensor_mul(
                                pt[:, lo:], pt[:, lo:], msk[:, idx, lo:])
                        pts.append(pt)
                        if (j > 0 or not causal) and t >= pogate:
                            po_step(2)
                        if t > 2:
                            emit_pv(t - 3)
                        if t % 2 == 1:
                            pr = qpool.tile([128, 512], BF, tag="pr", bufs=5)
                            nc.vector.tensor_add(pr[:], pts[t - 1][:],
                                                 pts[t][:])
                            gpairs.append(pr)
                        if t == groups[gi][1] - 1:
                            while len(gpairs) > 1:
                                nxt = []
                                for k in range(0, len(gpairs) - 1, 2):
                                    u = qpool.tile([128, 512], BF, tag="u",
                                                   bufs=3)
                                    nc.vector.tensor_add(
                                        u[:], gpairs[k][:], gpairs[k + 1][:])
                                    nxt.append(u)
                                if len(gpairs) % 2:
                                    nxt.append(gpairs[-1])
                                gpairs = nxt
                            nc.tensor.matmul(
                                lp["cur"][:], onehb[:, h, :], gpairs[0][:],
                                start=(h % 4 == 0 and gi == 0),
                                stop=(h % 4 == 3 and gi == len(groups) - 1))
                            gpairs = []
                            gi += 1
                    emit_pv(nkt - 3)
                    emit_pv(nkt - 2)
                    emit_pv(nkt - 1)
                    # raw evacuation on ScalarE (PSUM-fast port; DVE carries
                    # the pair-sum + mask load); normalized in place later
                    nc.scalar.copy(out=attnT[:, h, js], in_=outp[:])
                    # PE filler between heads covers the exp pipeline refill
                    po_step(16)
                emit_recip(1)
                for h in range(4, HL):
                    emit_tail(h, attnT[:, h, js])
                pending_po.extend(
                    (qt, nn) for nn in range(D // 512)
                    for qt in range(4 * j, 4 * j + 4))
            while pending_po or po_state["cur"] is not None:
                po_step(8)

    nc.compile()
    return nc


def _get_nc(mask_mode: str):
    if mask_mode not in _BUILD_CACHE:
        _BUILD_CACHE[mask_mode] = _build(mask_mode)
    return _BUILD_CACHE[mask_mode]


_DEINT = np.concatenate([np.arange(0, HD, 2), np.arange(1, HD, 2)])  # de-interleave


def _host_prep(x, freqs_cos, freqs_sin, mask, wq, wk, wv, wo):
    bf16 = ml_dtypes.bfloat16
    scale = float(HD) ** -0.5

    # mask mode
    mask = np.asarray(mask, np.float32)
    tril = np.tril(np.ones((S, S), bool))
    if np.all(mask == 0):
        mask_mode = "zero"
    elif np.all(mask[tril] == 0) and np.all(mask[~tril] <= -1e8):
        mask_mode = "causal"
    else:
        mask_mode = "general"

    # weights: de-interleave head dims of wq/wk; fold softmax scale into wq
    wq_p = (np.asarray(wq, np.float32).reshape(H, HD, D)[:, _DEINT, :] * scale)
    wk_p = np.asarray(wk, np.float32).reshape(KVH, HD, D)[:, _DEINT, :]
    wv_n = np.asarray(wv, np.float32).reshape(KVH, HD, D)
    wo_n = np.asarray(wo, np.float32)

    per_group = []
    for g in range(GROUPS):
        feats = np.concatenate([
            wq_p[g * HL:(g + 1) * HL].reshape(HL * HD, D),
            wk_p[g * KVL:(g + 1) * KVL].reshape(KVL * HD, D),
        ], axis=0)  # [1280, D]
        wqk_dma = np.ascontiguousarray(
            feats.reshape(FQK, 128, ND, 128).transpose(0, 3, 2, 1)).astype(bf16)
        wvg = wv_n[g * KVL:(g + 1) * KVL].reshape(KVL * HD, D)
        wv_dma = np.ascontiguousarray(
            wvg.reshape(KVL * HD, ND, 128).transpose(2, 1, 0)).astype(bf16)
        woT = wo_n[:, g * HL * HD:(g + 1) * HL * HD].T  # [1024, D]
        wo_dma = np.ascontiguousarray(
            woT.reshape(HL, 128, D).transpose(1, 0, 2)).astype(bf16)
        per_group.append((wqk_dma, wv_dma, wo_dma))

    xs = []
    for b in range(B):
        xT = np.asarray(x[b], np.float32).T  # [D, S]
        xs.append(np.ascontiguousarray(
            xT.reshape(ND, 128, S).transpose(1, 0, 2)).astype(bf16))

    cosT = np.asarray(freqs_cos, np.float32).T  # [64, S]
    sinT = np.asarray(freqs_sin, np.float32).T
    cos_dma = np.ascontiguousarray(np.concatenate([cosT, cosT], 0)).astype(bf16)
    sin_dma = np.ascontiguousarray(np.concatenate([sinT, sinT], 0)).astype(bf16)

    P = np.zeros((128, 128), np.float32)
    for r in range(64):
        P[r, 64 + r] = -1.0
        P[64 + r, r] = 1.0
    pm = np.ascontiguousarray(P.T).astype(bf16)

    # one-hot helpers for the softmax denominator: oneh[:, h, :] routes the
    # column sum into rows h and (h+4)%8 — the mirror keeps the half-chunk
    # accumulator's unused rows finite (reciprocal of an exact 0 row would
    # make NaN/inf that the 0-weights of the broadcast matmul still absorb
    # as 0*inf=NaN); oneb[:, h, :] has row h all-ones (broadcast 1/l row h
    # across partitions)
    eye44 = np.tile(np.eye(4, dtype=np.float32), (2, 2))  # m ≡ h (mod 4)
    oneh = np.ascontiguousarray(
        np.broadcast_to(eye44, (128, HL, HL))).astype(bf16)
    oneb = np.ascontiguousarray(
        np.broadcast_to(np.eye(HL, dtype=np.float32)[:, :, None],
                        (HL, HL, 128))).astype(bf16)

    # mask is applied multiplicatively after exp: P *= exp(mask)
    mask_extra = {}
    if mask_mode == "causal":
        # diagonal-block patterns are chunk-invariant: block (t=4j+i, j)
        # only depends on i
        mT = np.exp(np.minimum(mask.T, 0.0))
        md = np.empty((4, 128, 512), np.float32)
        for i in range(4):
            md[i] = mT[i * 128:(i + 1) * 128, 0:512]
        mask_extra["maskd"] = md.astype(bf16)
    elif mask_mode == "general":
        with np.errstate(over="ignore"):
            mask_extra["maskt"] = np.ascontiguousarray(
                np.exp(mask.T)).astype(bf16)

    in_maps = []
    for c in range(N_CORES):
        b, g = c // GROUPS, c % GROUPS
        wqk_dma, wv_dma, wo_dma = per_group[g]
        m = {"xt": xs[b], "wqk": wqk_dma, "wv": wv_dma, "wo": wo_dma,
             "cosd": cos_dma, "sind": sin_dma, "pm": pm,
             "oneh": oneh, "oneb": oneb}
        m.update(mask_extra)
        in_maps.append(m)
    return mask_mode, in_maps


def kernel(x, freqs_cos, freqs_sin, positions, mask, wq, wk, wv, wo,
           _want_profile=False):
    mask_mode, in_maps = _host_prep(x, freqs_cos, freqs_sin, mask, wq, wk, wv, wo)
    nc = _get_nc(mask_mode)
    res = run_bass_kernel_spmd(nc, in_maps, core_ids=list(range(N_CORES)),
                               trace=_want_profile)
    out = np.zeros((B, S, D), np.float32)
    for c in range(N_CORES):
        out[c // GROUPS] += res.results[c]["po"]
    if _want_profile:
        kernel.last_exec_time_ns = res.exec_time_ns
        kernel.last_results = res
    return out


# revision 27
# speedup vs baseline: 1.0811x; 1.0150x over previous
"""Trainium2 Bass kernel for GQA attention with RoPE (nn_Attention_21603685499660).

Shapes (hardcoded): x [2, 2048, 4096], H=32 Q heads, KVH=8 KV heads, HD=128.
Sharding over 8 NeuronCores: core c -> batch b = c//4, head-group g = c%4
(8 Q heads, 2 KV heads per core).  Each core computes a partial output
(its heads' attention output through its slice of wo); the host sums the
4 partials per batch.  No on-device collectives.

Per-core pipeline (all matmuls bf16 with f32 PSUM accumulation):
  1. QKV projection from host-pre-transposed x and weights.  Q/K are
     produced directly in transposed [HD, seq] layout; V in natural
     [seq, HD] layout.  RoPE applied via a rotate-half permutation
     matmul (head dims de-interleaved host-side).  For the causal case,
     attention chunk 0 (k-tiles 0-3, available after stage-1 chunk 0)
     is interleaved into stage-1 chunks 1-3 — one QK/exp/PV tile
     iteration after each projection group — so its ACT-bound work
     rides under stage-1's dense PE stream instead of stalling the PE
     (and HAM-rethrottling the clock) at stage-2 entry.  Its pair-sums
     are stored (qds) and its softmax denominators finalized at stage-2
     entry.
  2. Attention with scores computed transposed: ST[k,q] = K @ Q^T per
     (head, 512-wide q chunk, 128-wide k tile).  Softmax without max
     subtraction (scores are O(+-10)): P = exp(ST); causal masking is
     multiplicative 0/1 on the P tile in bf16, and the diagonal blocks
     compute QK/exp/PV only on their unmasked column sub-range
     (q >= 128*i for pattern i).  The denominator l accumulates on
     TensorE into an [8,512] PSUM tile (one-hot stationary routes the
     column sum to rows h and h+4; the mirror keeps unused rows finite)
     split heads 0-3 / 4-7 per chunk; 1/l is one DVE
     reciprocal_approx_fast per half (no Ln -> single activation table
     set for the whole kernel).  1/l row h is partition-broadcast by a
     one-hot-row matmul; raw PV output is evacuated on ScalarE and
     normalized in place once its half's reciprocal is ready.
  3. Output projection po[q,n] += attnT[d,q]^T @ woT[d,n], emitted as
     PE filler interleaved into the NEXT chunk's attention.  wo is
     streamed nn-major and pending groups are consumed nn-major so the
     first groups never wait on the 8.4MB wo DMA.
"""

from contextlib import ExitStack

import numpy as np
import ml_dtypes

import concourse.bass as bass
import concourse.tile as tile
from concourse import bacc, mybir
from concourse.bass_utils import run_bass_kernel_spmd

B, S, D = 2, 2048, 4096
H, KVH, HD = 32, 8, 128
N_CORES = 8
GROUPS = 4            # head groups (tensor-parallel dim); B * GROUPS = 8 cores
HL = H // GROUPS      # 8 local Q heads
KVL = KVH // GROUPS   # 2 local KV heads
FQK = HL + KVL        # 10 feature tiles of 128 (Q heads then K heads)
NJ = S // 512         # 4 seq chunks of 512
NT = S // 128         # 16 seq tiles of 128
ND = D // 128         # 32 contraction tiles
BF = mybir.dt.bfloat16
F32 = mybir.dt.float32
EXP = mybir.ActivationFunctionType.Exp

_BUILD_CACHE: dict = {}


def _build(mask_mode: str):
    """mask_mode: 'causal' | 'zero' | 'general'."""
    causal = mask_mode == "causal"
    nc = bacc.Bacc("TRN2", target_bir_lowering=False, debug=False,
                   num_devices=N_CORES)

    xt_d = nc.dram_tensor("xt", [128, ND, S], BF, kind="ExternalInput").ap()
    wqk_d = nc.dram_tensor("wqk", [FQK, 128, ND, 128], BF, kind="ExternalInput").ap()
    wv_d = nc.dram_tensor("wv", [128, ND, KVL * HD], BF, kind="ExternalInput").ap()
    wo_d = nc.dram_tensor("wo", [128, HL, D], BF, kind="ExternalInput").ap()
    cos_d = nc.dram_tensor("cosd", [128, S], BF, kind="ExternalInput").ap()
    sin_d = nc.dram_tensor("sind", [128, S], BF, kind="ExternalInput").ap()
    pm_d = nc.dram_tensor("pm", [128, 128], BF, kind="ExternalInput").ap()
    oneh_d = nc.dram_tensor("oneh", [128, HL, HL], BF, kind="ExternalInput").ap()
    oneb_d = nc.dram_tensor("oneb", [HL, HL, 128], BF, kind="ExternalInput").ap()
    if causal:
        # the 4 diagonal-block 0/1 patterns are chunk-invariant
        mk_d = nc.dram_tensor("maskd", [4, 128, 512], BF, kind="ExternalInput").ap()
    elif mask_mode == "general":
        mk_d = nc.dram_tensor("maskt", [S, S], BF, kind="ExternalInput").ap()
    po_d = nc.dram_tensor("po", [S, D], F32, kind="ExternalOutput").ap()

    def apply_tiles(j):
        if causal:
            return list(range(4 * j, 4 * j + 4))
        if mask_mode == "general":
            return list(range(NT))
        return []

    with tile.TileContext(nc) as tc, ExitStack() as ctx:
        resident = ctx.enter_context(tc.tile_pool(name="resident", bufs=1))
        qkv = ctx.enter_context(tc.tile_pool(name="qkv", bufs=1))

        ones_col = resident.tile([128, 1], BF)
        nc.vector.memset(ones_col[:], 1.0)
        pmb = resident.tile([128, 128], BF)
        nc.sync.dma_start(out=pmb[:], in_=pm_d[:])
        onehb = resident.tile([128, HL, HL], BF)   # l-accum one-hot columns
        nc.sync.dma_start(out=onehb[:], in_=oneh_d[:])
        onebb = resident.tile([HL, HL, 128], BF)   # 1/l bcast one-hot rows
        nc.sync.dma_start(out=onebb[:], in_=oneb_d[:])
        if causal:
            mskr = resident.tile([128, 4, 512], BF)
            for i in range(4):
                nc.sync.dma_start(out=mskr[:, i, :], in_=mk_d[i])
        # preload the Exp activation table set while the first DMAs land;
        # no other set is ever used, so this is the kernel's only
        # ACT_TABLE_LOAD (copy lives in every set)
        dume = resident.tile([128, 1], BF)
        nc.scalar.activation(out=dume[:], in_=ones_col[:], func=EXP)
        wmv = resident.tile([128, 128], BF)   # warm-up moving operand
        nc.vector.memset(wmv[:], 0.0)

        QT = qkv.tile([128, HL, S], BF)    # [HD, head, seq] (de-interleaved)
        KT = qkv.tile([128, KVL, S], BF)
        V = qkv.tile([128, NT, KVL * HD], BF)  # [seq%128, seqtile, kv-head*HD]
        if causal:
            attn0 = qkv.tile([128, HL, 512], BF)   # chunk-0 raw attention out
            qds = qkv.tile([128, HL, 512], BF)     # chunk-0 P quad-sums

        # ---- stage 1: QKV projection + RoPE (+ causal chunk-0 attention) ----
        with tc.tile_pool(name="s1const", bufs=1) as s1const, \
             tc.tile_pool(name="xpool", bufs=2) as xpool, \
             tc.tile_pool(name="wpool", bufs=2) as wpool, \
             tc.tile_pool(name="tpool", bufs=3) as tpool, \
             tc.tile_pool(name="p0pool", bufs=3) as p0pool, \
             tc.tile_pool(name="ps_qk", bufs=2, space="PSUM") as ps_qk, \
             tc.tile_pool(name="ps_ro", bufs=1, space="PSUM") as ps_ro, \
             tc.tile_pool(name="ps_v", bufs=2, space="PSUM") as ps_v, \
             tc.tile_pool(name="ps_c0", bufs=1, space="PSUM") as ps_c0, \
             tc.tile_pool(name="ps_c0o", bufs=1, space="PSUM") as ps_c0o:
            cosb = s1const.tile([128, S], BF)
            sinb = s1const.tile([128, S], BF)
            wvb = s1const.tile([128, ND, KVL * HD], BF)
            # PE warm-up through the HAM window while the first DMAs land;
            # memset moving operand so it depends on no DMA.
            for _ in range(48):
                wtile = ps_ro.tile([1, 128], F32, tag="warm")
                nc.tensor.matmul(wtile[:], ones_col[:], wmv[:], start=True, stop=True)

            # -- causal chunk-0 attention, one tile-iteration per call,
            # interleaved into stage-1 chunks 1-3 (32 iterations total) --
            c0 = {"s": 0, "outp": None, "pts": None, "prs": None}

            def c0_pv(t):
                lo = 128 * t
                nc.tensor.matmul(c0["outp"][:, lo:],
                                 V[:, t, bass.ts(c0["s"] // 4 // 4, 128)],
                                 c0["pts"][t][:, lo:],
                                 start=(t == 0), stop=(t == 3))

            def c0_step():
                s = c0["s"]
                if not causal or s >= 32:
                    return
                h, t = s // 4, s % 4
                hk = h // 4
                lo = 128 * t
                if t == 0:
                    c0["outp"] = ps_c0o.tile([128, 512], F32, tag="o0",
                                             name="outp0")
                    c0["pts"], c0["prs"] = [], []
                stp0 = ps_c0.tile([128, 512], F32, tag="st0", name="stp0")
                nc.tensor.matmul(stp0[:, lo:], KT[:, hk, bass.ts(t, 128)],
                                 QT[:, h, lo:512], start=True, stop=True)
                if t > 0:
                    c0_pv(t - 1)
                pt0 = p0pool.tile([128, 512], BF, tag="pt0")
                if lo:
                    nc.vector.memset(pt0[:, :lo], 0.0)
                nc.scalar.activation(out=pt0[:, lo:], in_=stp0[:, lo:], func=EXP)
                nc.vector.tensor_mul(pt0[:, lo:], pt0[:, lo:], mskr[:, t, lo:])
                c0["pts"].append(pt0)
                if t % 2 == 1:
                    pr = p0pool.tile([128, 512], BF, tag="pr0", bufs=2)
                    nc.vector.tensor_add(pr[:], c0["pts"][t - 1][:],
                                         c0["pts"][t][:])
                    c0["prs"].append(pr)
                if t == 3:
                    nc.vector.tensor_add(qds[:, h, :], c0["prs"][0][:],
                                         c0["prs"][1][:])
                    c0_pv(3)
                    nc.scalar.copy(out=attn0[:, h, :], in_=c0["outp"][:])
                c0["s"] += 1

            def rope_emit(ent):
                # deferred RoPE for a finished projection group: done one
                # group later so its pq matmul / DVE work never stalls PE
                raw, f, js = ent
                pq = ps_ro.tile([128, 512], F32, tag="ro")
                nc.tensor.matmul(pq[:], pmb[:], raw[:], start=True, stop=True)
                t1 = tpool.tile([128, 512], BF, tag="t1")
                nc.vector.tensor_mul(t1[:], raw[:], cosb[:, js])
                t2 = tpool.tile([128, 512], BF, tag="t2")
                nc.vector.tensor_mul(t2[:], pq[:], sinb[:, js])
                dest = QT[:, f, js] if f < HL else KT[:, f - HL, js]
                nc.vector.tensor_add(dest, t1[:], t2[:])

            # chunk 0's x in 4 sub-DMAs (first 8 K-tiles land fast); later
            # chunks prefetched as one DMA during the previous chunk
            wf_next = wpool.tile([128, ND, 128], BF, tag="wf")
            nc.sync.dma_start(out=wf_next[:], in_=wqk_d[0])
            xj_cur = xpool.tile([128, ND, 512], BF, tag="xj", name="xj0")
            for q in range(4):
                nc.sync.dma_start(out=xj_cur[:, bass.ts(q, 8), :],
                                  in_=xt_d[:, bass.ts(q, 8), bass.ts(0, 512)])
            for j in range(NJ):
                js = bass.ts(j, 512)
                xj = xj_cur
                for f in range(FQK):
                    wf = wf_next
                    # prefetch the next group's weights one group ahead
                    nf = f + 1 if f + 1 < FQK else 0
                    if f + 1 < FQK or j + 1 < NJ:
                        wf_next = wpool.tile([128, ND, 128], BF, tag="wf")
                        nc.sync.dma_start(out=wf_next[:], in_=wqk_d[nf])
                    else:
                        wf_next = None
                    if j == 0 and f == 0:
                        nc.sync.dma_start(out=cosb[:], in_=cos_d[:])
                        nc.sync.dma_start(out=sinb[:], in_=sin_d[:])
                    if j == 0 and f in (6, 7):
                        # wv halves late enough not to delay the wf
                        # prefetch chain; needed only by the j=0 V groups
                        half = bass.ts(f - 6, ND // 2)
                        nc.sync.dma_start(out=wvb[:, half, :],
                                          in_=wv_d[:, half, :])
                    if f in (1, 4) and j + 1 < NJ:
                        # next chunk's x in two halves so the wf prefetches
                        # in between are not delayed behind 4.2MB
                        if f == 1:
                            xj_cur = xpool.tile([128, ND, 512], BF, tag="xj",
                                                name="xjn")
                        half = bass.ts(f // 3, ND // 2)
                        nc.sync.dma_start(
                            out=xj_cur[:, half, :],
                            in_=xt_d[:, half, bass.ts(j + 1, 512)])
                    ps = ps_qk.tile([128, 512], F32, tag="qk")
                    for n in range(ND):
                        nc.tensor.matmul(ps[:], wf[:, n, :], xj[:, n, :],
                                         start=(n == 0), stop=(n == ND - 1))
                    raw = tpool.tile([128, 512], BF, tag="raw")
                    nc.scalar.copy(out=raw[:], in_=ps[:])
                    rope_emit((raw, f, js))
                    if j >= 1:
                        c0_step()
                for tt in range(4):
                    psv = ps_v.tile([128, KVL * HD], F32, tag="v")
                    for n in range(ND):
                        nc.tensor.matmul(psv[:], xj[:, n, bass.ts(tt, 128)],
                                         wvb[:, n, :],
                                         start=(n == 0), stop=(n == ND - 1))
                    nc.scalar.copy(out=V[:, j * 4 + tt, :], in_=psv[:])
                    if j >= 1:
                        c0_step()

        # attnT + wo live from stage 2 through stage 3 (pool opened only now
        # so stage 1 had the SBUF).
        att_out = ctx.enter_context(tc.tile_pool(name="att_out", bufs=1))
        attnT = att_out.tile([128, HL, S], BF)  # [HD, head, seq]
        wob = att_out.tile([128, HL, D], BF)

        # ---- stage 2+3: attention chunks (1-3 for causal) + out-projection --
        po_state = {"cur": None, "dd": 0}

        def po_step(budget):
            # emit up to `budget` output-projection matmuls as PE filler
            for _ in range(budget):
                if po_state["cur"] is None:
                    if not pending_po:
                        return
                    qt, nn = pending_po.pop(0)
                    pop = ps_po.tile([128, 512], F32, tag="po", name="pop")
                    po_state["cur"] = (qt, nn, pop)
                    po_state["dd"] = 0
                qt, nn, pop = po_state["cur"]
                dd = po_state["dd"]
                src = (attn0[:, dd, bass.ts(qt, 128)] if causal and qt < 4
                       else attnT[:, dd, bass.ts(qt, 128)])
                nc.tensor.matmul(pop[:], src, wob[:, dd, bass.ts(nn, 512)],
                                 start=(dd == 0), stop=(dd == HL - 1))
                po_state["dd"] += 1
                if po_state["dd"] == HL:
                    stg = spool.tile([128, 512], F32, tag="stg")
                    # alternate evacuation engine to balance ACT/DVE load
                    if (qt + nn) % 2:
                        nc.scalar.copy(out=stg[:], in_=pop[:])
                    else:
                        nc.vector.tensor_copy(stg[:], pop[:])
                    nc.sync.dma_start(
                        out=po_d[bass.ts(qt, 128), bass.ts(nn, 512)], in_=stg[:])
                    po_state["cur"] = None

        with tc.tile_pool(name="mpool", bufs=1) as mpool, \
             tc.tile_pool(name="ppool", bufs=6) as ppool, \
             tc.tile_pool(name="qpool", bufs=4) as qpool, \
             tc.tile_pool(name="npool", bufs=2) as npool, \
             tc.tile_pool(name="spool", bufs=3) as spool, \
             tc.tile_pool(name="ps_st", bufs=2, space="PSUM") as ps_st, \
             tc.tile_pool(name="ps_o", bufs=2, space="PSUM") as ps_o, \
             tc.tile_pool(name="ps_l", bufs=1, space="PSUM") as ps_l, \
             tc.tile_pool(name="ps_rb", bufs=1, space="PSUM") as ps_rb, \
             tc.tile_pool(name="ps_po", bufs=2, space="PSUM") as ps_po:
            pending_po = []  # (qt, nn) groups ready to emit as PE filler
            lp = {"cur": None}
            rlh = {}

            def emit_recip(g):
                lr32 = npool.tile([HL, 512], F32, tag="lr", name="lr32")
                nc.vector.reciprocal_approx_fast(out=lr32[:], in_=lp["cur"][:])
                rl = npool.tile([HL, 512], BF, tag="rl", name="rl")
                nc.vector.tensor_copy(rl[:], lr32[:])
                rlh[g] = rl
                lp["cur"] = None

            def emit_tail(h, dest):
                # broadcast row h of 1/l across partitions (one-hot row
                # matmul) and normalize the raw attention output in place
                rbp = ps_rb.tile([128, 512], F32, tag="rb", name="rbp")
                nc.tensor.matmul(rbp[:], onebb[:, h, :], rlh[h // 4][:],
                                 start=True, stop=True)
                nc.vector.tensor_mul(dest, dest, rbp[:])

            # wo streamed nn-major so the first (nn-major-ordered) po groups
            # never wait on the full 8.4MB transfer
            for nn in range(D // 512):
                nc.sync.dma_start(out=wob[:, :, bass.ts(nn, 512)],
                                  in_=wo_d[:, :, bass.ts(nn, 512)])

            if causal:
                # finalize chunk 0: denominators from the stored quad-sums,
                # then normalize attn0 and queue its output projection
                for g in range(2):
                    lp["cur"] = ps_l.tile([HL, 512], F32, tag="l", name="lp")
                    for hh in range(4):
                        h = 4 * g + hh
                        nc.tensor.matmul(lp["cur"][:], onehb[:, h, :],
                                         qds[:, h, :],
                                         start=(hh == 0), stop=(hh == 3))
                    emit_recip(g)
                for h in range(HL):
                    emit_tail(h, attn0[:, h, :])
                pending_po.extend(
                    (qt, nn) for nn in range(D // 512) for qt in range(4))

            for j in range(1 if causal else 0, NJ):
                js = bass.ts(j, 512)
                nkt = 4 * (j + 1) if causal else NT
                atiles = apply_tiles(j)
                if causal:
                    msk = mskr
                elif mask_mode == "general":
                    msk = mpool.tile([128, NT, 512], BF, tag="msk")
                    for idx, t in enumerate(atiles):
                        nc.sync.dma_start(out=msk[:, idx, :],
                                          in_=mk_d[bass.ts(t, 128), js])
                # l accumulation groups of 8 k-tiles (remainder 4): pair-sums
                # on DVE as tiles arrive, tree-combine + one l matmul per group
                if nkt <= 8:
                    groups = [(0, nkt)]
                else:
                    groups = [(0, 8), (8, nkt)]
                los = [0] * nkt
                if causal:
                    for i in range(1, 4):
                        los[4 * j + i] = 128 * i
                first_j = j == (1 if causal else 0)
                for h in range(HL):
                    hk = h // (HL // KVL)
                    if h == 4:
                        emit_recip(0)
                    if 4 <= h:
                        # tails for heads 0-3 spread over heads 4-7
                        emit_tail(h - 4, attnT[:, h - 4, js])
                    if lp["cur"] is None:
                        lp["cur"] = ps_l.tile([HL, 512], F32, tag="l",
                                              name="lp")
                    outp = ps_o.tile([128, 512], F32, tag="out")
                    pts = []
                    # software pipeline: PV_t emitted two tiles after QK_t so
                    # QK + filler sit in the PE stream while exp_t (+mask) runs
                    def emit_pv(t):
                        lo = los[t]
                        nc.tensor.matmul(outp[:, lo:], V[:, t, bass.ts(hk, 128)],
                                         pts[t][:, lo:],
                                         start=(t == 0), stop=(t == nkt - 1))

                    gi = 0
                    gpairs = []
                    pogate = 6 if (causal and first_j) else 2
                    for t in range(nkt):
                        # causal diagonal block i only has unmasked columns
                        # >= 128*i: compute QK/exp/PV on that sub-range and
                        # zero the rest of the P tile
                        lo = los[t]
                        stp = ps_st.tile([128, 512], F32, tag="st")
                        nc.tensor.matmul(stp[:, lo:], KT[:, hk, bass.ts(t, 128)],
                                         QT[:, h, j * 512 + lo:(j + 1) * 512],
                                         start=True, stop=True)
                        pt = ppool.tile([128, 512], BF, tag="pt")
                        if lo:
                            nc.vector.memset(pt[:, :lo], 0.0)
                        nc.scalar.activation(out=pt[:, lo:], in_=stp[:, lo:],
                                             func=EXP)
                        if t in atiles:
                            # multiplicative mask exp(m): 0/1 for causal
                            idx = atiles.index(t)
                            nc.vector.tensor_mul(
                                pt[:, lo:], pt[:, lo:], msk[:, idx, lo:])
                        pts.append(pt)
                        if (j > 0 or not causal) and t >= pogate:
                            po_step(2)
                        if t > 1:
                            emit_pv(t - 2)
                        if t % 2 == 1:
                            pr = qpool.tile([128, 512], BF, tag="pr", bufs=5)
                            nc.vector.tensor_add(pr[:], pts[t - 1][:],
                                                 pts[t][:])
                            gpairs.append(pr)
                        if t == groups[gi][1] - 1:
                            while len(gpairs) > 1:
                                nxt = []
                                for k in range(0, len(gpairs) - 1, 2):
                                    u = qpool.tile([128, 512], BF, tag="u",
                                                   bufs=3)
                                    nc.vector.tensor_add(
                                        u[:], gpairs[k][:], gpairs[k + 1][:])
                                    nxt.append(u)
                                if len(gpairs) % 2:
                                    nxt.append(gpairs[-1])
                                gpairs = nxt
                            nc.tensor.matmul(
                                lp["cur"][:], onehb[:, h, :], gpairs[0][:],
                                start=(h % 4 == 0 and gi == 0),
                                stop=(h % 4 == 3 and gi == len(groups) - 1))
                            gpairs = []
                            gi += 1
                    emit_pv(nkt - 2)
                    emit_pv(nkt - 1)
                    # raw evacuation on ScalarE (PSUM-fast port; DVE carries
                    # the pair-sum + mask load); normalized in place later
                    nc.scalar.copy(out=attnT[:, h, js], in_=outp[:])
                    # PE filler between heads covers the exp pipeline refill
                    po_step(16)
                emit_recip(1)
                for h in range(4, HL):
                    emit_tail(h, attnT[:, h, js])
                pending_po.extend(
                    (qt, nn) for nn in range(D // 512)
                    for qt in range(4 * j, 4 * j + 4))
            while pending_po or po_state["cur"] is not None:
                po_step(8)

    nc.compile()
    return nc


def _get_nc(mask_mode: str):
    if mask_mode not in _BUILD_CACHE:
        _BUILD_CACHE[mask_mode] = _build(mask_mode)
    return _BUILD_CACHE[mask_mode]


_DEINT = np.concatenate([np.arange(0, HD, 2), np.arange(1, HD, 2)])  # de-interleave


def _host_prep(x, freqs_cos, freqs_sin, mask, wq, wk, wv, wo):
    bf16 = ml_dtypes.bfloat16
    scale = float(HD) ** -0.5

    # mask mode
    mask = np.asarray(mask, np.float32)
    tril = np.tril(np.ones((S, S), bool))
    if np.all(mask == 0):
        mask_mode = "zero"
    elif np.all(mask[tril] == 0) and np.all(mask[~tril] <= -1e8):
        mask_mode = "causal"
    else:
        mask_mode = "general"

    # weights: de-interleave head dims of wq/wk; fold softmax scale into wq
    wq_p = (np.asarray(wq, np.float32).reshape(H, HD, D)[:, _DEINT, :] * scale)
    wk_p = np.asarray(wk, np.float32).reshape(KVH, HD, D)[:, _DEINT, :]
    wv_n = np.asarray(wv, np.float32).reshape(KVH, HD, D)
    wo_n = np.asarray(wo, np.float32)

    per_group = []
    for g in range(GROUPS):
        feats = np.concatenate([
            wq_p[g * HL:(g + 1) * HL].reshape(HL * HD, D),
            wk_p[g * KVL:(g + 1) * KVL].reshape(KVL * HD, D),
        ], axis=0)  # [1280, D]
        wqk_dma = np.ascontiguousarray(
            feats.reshape(FQK, 128, ND, 128).transpose(0, 3, 2, 1)).astype(bf16)
        wvg = wv_n[g * KVL:(g + 1) * KVL].reshape(KVL * HD, D)
        wv_dma = np.ascontiguousarray(
            wvg.reshape(KVL * HD, ND, 128).transpose(2, 1, 0)).astype(bf16)
        woT = wo_n[:, g * HL * HD:(g + 1) * HL * HD].T  # [1024, D]
        wo_dma = np.ascontiguousarray(
            woT.reshape(HL, 128, D).transpose(1, 0, 2)).astype(bf16)
        per_group.append((wqk_dma, wv_dma, wo_dma))

    xs = []
    for b in range(B):
        xT = np.asarray(x[b], np.float32).T  # [D, S]
        xs.append(np.ascontiguousarray(
            xT.reshape(ND, 128, S).transpose(1, 0, 2)).astype(bf16))

    cosT = np.asarray(freqs_cos, np.float32).T  # [64, S]
    sinT = np.asarray(freqs_sin, np.float32).T
    cos_dma = np.ascontiguousarray(np.concatenate([cosT, cosT], 0)).astype(bf16)
    sin_dma = np.ascontiguousarray(np.concatenate([sinT, sinT], 0)).astype(bf16)

    P = np.zeros((128, 128), np.float32)
    for r in range(64):
        P[r, 64 + r] = -1.0
        P[64 + r, r] = 1.0
    pm = np.ascontiguousarray(P.T).astype(bf16)

    # one-hot helpers for the softmax denominator: oneh[:, h, :] routes the
    # column sum into rows h and (h+4)%8 — the mirror keeps the half-chunk
    # accumulator's unused rows finite (reciprocal of an exact 0 row would
    # make NaN/inf that the 0-weights of the broadcast matmul still absorb
    # as 0*inf=NaN); oneb[:, h, :] has row h all-ones (broadcast 1/l row h
    # across partitions)
    eye44 = np.tile(np.eye(4, dtype=np.float32), (2, 2))  # m ≡ h (mod 4)
    oneh = np.ascontiguousarray(
        np.broadcast_to(eye44, (128, HL, HL))).astype(bf16)
    oneb = np.ascontiguousarray(
        np.broadcast_to(np.eye(HL, dtype=np.float32)[:, :, None],
                        (HL, HL, 128))).astype(bf16)

    # mask is applied multiplicatively after exp: P *= exp(mask)
    mask_extra = {}
    if mask_mode == "causal":
        # diagonal-block patterns are chunk-invariant: block (t=4j+i, j)
        # only depends on i
        mT = np.exp(np.minimum(mask.T, 0.0))
        md = np.empty((4, 128, 512), np.float32)
        for i in range(4):
            md[i] = mT[i * 128:(i + 1) * 128, 0:512]
        mask_extra["maskd"] = md.astype(bf16)
    elif mask_mode == "general":
        with np.errstate(over="ignore"):
            mask_extra["maskt"] = np.ascontiguousarray(
                np.exp(mask.T)).astype(bf16)

    in_maps = []
    for c in range(N_CORES):
        b, g = c // GROUPS, c % GROUPS
        wqk_dma, wv_dma, wo_dma = per_group[g]
        m = {"xt": xs[b], "wqk": wqk_dma, "wv": wv_dma, "wo": wo_dma,
             "cosd": cos_dma, "sind": sin_dma, "pm": pm,
             "oneh": oneh, "oneb": oneb}
        m.update(mask_extra)
        in_maps.append(m)
    return mask_mode, in_maps


def kernel(x, freqs_cos, freqs_sin, positions, mask, wq, wk, wv, wo,
           _want_profile=False):
    mask_mode, in_maps = _host_prep(x, freqs_cos, freqs_sin, mask, wq, wk, wv, wo)
    nc = _get_nc(mask_mode)
    res = run_bass_kernel_spmd(nc, in_maps, core_ids=list(range(N_CORES)),
                               trace=_want_profile)
    out = np.zeros((B, S, D), np.float32)
    for c in range(N_CORES):
        out[c // GROUPS] += res.results[c]["po"]
    if _want_profile:
        kernel.last_exec_time_ns = res.exec_time_ns
        kernel.last_results = res
    return out
